# revision 11
# baseline (speedup 1.0000x reference)
"""Deformable Conv2d (DCNv2) Trainium2 Bass kernel.

Sharding: 8 cores; core c handles batch b = c//2, output-row half c%2
(48 of 96 rows). Each core receives a zero-padded window of its batch's
input (60 rows x 108 cols, pad 6 each side) so all bilinear samples and
the aux 3x3 convs are core-local.

Column ordering: the aux pipeline (aux conv, offsets, corner weights,
flat gather indices) runs in natural raster order n'. GPSIMD ap_gather
consumes indices "wrapped" over 16 partitions (output column i takes
the index at partition i%16, slot i//16), so the gather/combine/main-
matmul stage runs in wrapped order j, where within a 864-column chunk
j = 16*s + p corresponds to n'_local = 54*p + s. Corner weights are
written through a wrap-permuting access pattern, index tiles are built
with contiguous-run DMAs + a doubling ladder, and outputs are unwrapped
with one strided copy before the store DMA.

Per-tap corner weights are broadcast to all 128 channel partitions with
one gpsimd partition_broadcast (instead of a serialized ladder of
doubling DMAs), the combined bilinear values are written as bf16, and
the main + aux convolutions run as bf16 matmuls (1 cycle/row vs fp32's
4). Main-PSUM eviction runs on the vector engine so the scalar queue
never blocks the next chunk's aux activations.
"""

import os
import sys
from contextlib import ExitStack

import numpy as np
from ml_dtypes import bfloat16

if "/opt/trn_rl_repo" not in sys.path:
    sys.path.insert(0, "/opt/trn_rl_repo")

import concourse.bass as bass
import concourse.bacc as bacc
import concourse.mybir as mybir
import concourse.tile as tile
from concourse.bass_utils import run_bass_kernel_spmd

F32 = mybir.dt.float32
BF16 = mybir.dt.bfloat16
I16 = mybir.dt.int16
I32 = mybir.dt.int32
ALU = mybir.AluOpType
ACTF = mybir.ActivationFunctionType

# problem shape (hardcoded)
B, C, CO, H, W = 4, 128, 256, 96, 96
KK = 9
PAD = 6               # window pad on each side
HR = 48               # output rows per core
ROWS = HR + 2 * PAD   # 60 window rows
PITCH = W + 2 * PAD   # 108
XWN = ROWS * PITCH    # 6480 window elems
NP = HR * PITCH       # 5184 pipeline columns (with junk cols)
SW = 54               # wrapped idx slots per gather call
CHUNK = 16 * SW       # 864
NCHUNK = NP // CHUNK  # 6
SUB = 432             # matmul N-tile
SUBS = CHUNK // SUB   # 2
RPC = CHUNK // PITCH  # 8 output rows per chunk
IDX_BASE = PAD * PITCH  # 648 (w_w already includes the column pad)
CORNER_OFF = (0, 1, PITCH, PITCH + 1)

_CACHE: dict = {}


def _conv_off(ky, kx):
    # window-flat offset of conv tap (ky,kx) relative to output column n'
    return (PAD - 1 + ky) * PITCH + (kx - 1)


def _build_program():
    nc = bacc.Bacc(
        "TRN2",
        target_bir_lowering=False,
        debug=False,
        enable_asserts=False,
        num_devices=1,
    )
    d_xw = nc.dram_tensor("xw", [C, XWN], F32, kind="ExternalInput").ap()
    d_xwb = nc.dram_tensor("xwb", [C, XWN], BF16, kind="ExternalInput").ap()
    d_wmain = nc.dram_tensor("wmain", [C, KK * CO], BF16, kind="ExternalInput").ap()
    d_waux = nc.dram_tensor("waux", [C, KK * 27], BF16, kind="ExternalInput").ap()
    d_baux = nc.dram_tensor("baux", [27, 1], F32, kind="ExternalInput").ap()
    d_bmain = nc.dram_tensor("bmain", [128, 2], F32, kind="ExternalInput").ap()
    d_sprime = nc.dram_tensor("sprime", [128, KK * SW], F32, kind="ExternalInput").ap()
    d_out = nc.dram_tensor("out", [CO, HR, W], F32, kind="ExternalOutput").ap()

    NIW = KK * SW  # idx cols per corner per chunk (486)

    with tile.TileContext(nc) as tc, ExitStack() as ctx:
        cpool = ctx.enter_context(tc.tile_pool(name="consts", bufs=1))
        t_xw = cpool.tile([C, XWN], F32, tag="xw")
        nc.sync.dma_start(t_xw[:], d_xw)
        t_xwb = cpool.tile([C, XWN], BF16, tag="xwb")
        nc.sync.dma_start(t_xwb[:], d_xwb)
        t_wmain = cpool.tile([C, KK * CO], BF16, tag="wmain")
        nc.sync.dma_start(t_wmain[:], d_wmain)
        t_waux = cpool.tile([C, KK * 27], BF16, tag="waux")
        nc.sync.dma_start(t_waux[:], d_waux)
        t_baux = cpool.tile([27, 1], F32, tag="baux")
        nc.sync.dma_start(t_baux[:], d_baux)
        t_bmain = cpool.tile([128, 2], F32, tag="bmain")
        nc.sync.dma_start(t_bmain[:], d_bmain)
        t_sprime = cpool.tile([128, NIW], F32, tag="sprime")
        nc.sync.dma_start(t_sprime[:], d_sprime)

        apool = ctx.enter_context(tc.tile_pool(name="auxp", bufs=2, space="PSUM"))
        mpool = ctx.enter_context(tc.tile_pool(name="mainp", bufs=1, space="PSUM"))
        auxf_pool = ctx.enter_context(tc.tile_pool(name="auxf", bufs=2))
        spool = ctx.enter_context(tc.tile_pool(name="scratch", bufs=1))
        wrow_pool = ctx.enter_context(tc.tile_pool(name="wrows", bufs=1))
        wbpool = ctx.enter_context(tc.tile_pool(name="wb", bufs=1))
        wstg_pool = ctx.enter_context(tc.tile_pool(name="wstg", bufs=2))
        ipool = ctx.enter_context(tc.tile_pool(name="idx", bufs=2))
        gpool = ctx.enter_context(tc.tile_pool(name="gath", bufs=2))
        vpool = ctx.enter_context(tc.tile_pool(name="val", bufs=2))
        opool = ctx.enter_context(tc.tile_pool(name="outsb", bufs=1))
        upool = ctx.enter_context(tc.tile_pool(name="unw", bufs=1))

        for t in range(NCHUNK):
            cbase = t * CHUNK

            # ---- aux conv: 27 channels over this chunk, natural order ----
            aux27 = auxf_pool.tile([27, CHUNK], F32, tag="aux27", bufs=1)
            for u in range(SUBS):
                pa = apool.tile([27, SUB], F32, tag="auxpsum")
                for k in range(KK):
                    ky, kx = k // 3, k % 3
                    base = cbase + u * SUB + _conv_off(ky, kx)
                    nc.tensor.matmul(
                        pa[:],
                        t_waux[:, k * 27 : (k + 1) * 27],
                        t_xwb[:, base : base + SUB],
                        start=(k == 0),
                        stop=(k == KK - 1),
                    )
                nc.scalar.activation(
                    aux27[:, u * SUB : (u + 1) * SUB],
                    pa[:],
                    ACTF.Identity,
                    bias=t_baux[:, 0:1],
                )
            # regroup the three 9-row bands side by side on partitions 0-8
            auxf = auxf_pool.tile([9, 3 * CHUNK], F32, tag="auxf")
            nc.sync.dma_start(auxf[:, 0:CHUNK], aux27[0:9, :])
            nc.sync.dma_start(auxf[:, CHUNK : 2 * CHUNK], aux27[9:18, :])
            nc.sync.dma_start(auxf[:, 2 * CHUNK : 3 * CHUNK], aux27[18:27, :])
            # mask = sigmoid(logit), in place at partition base 0
            nc.scalar.activation(
                auxf[:, 2 * CHUNK : 3 * CHUNK],
                auxf[:, 2 * CHUNK : 3 * CHUNK],
                ACTF.Sigmoid,
            )

            # ---- floor(ry), floor(rx); fractional parts ----
            c32 = spool.tile([9, 2 * CHUNK], I32, tag="c32")
            nc.vector.tensor_copy(c32[:], auxf[:, 0 : 2 * CHUNK])
            r0f = spool.tile([9, 2 * CHUNK], F32, tag="r0f")
            nc.vector.tensor_copy(r0f[:], c32[:])
            gt = spool.tile([9, 2 * CHUNK], F32, tag="c32")  # reuse slot
            nc.vector.tensor_tensor(gt[:], r0f[:], auxf[:, 0 : 2 * CHUNK], ALU.is_gt)
            # r0f <- floor = round - (round > x)
            nc.vector.tensor_tensor(r0f[:], r0f[:], gt[:], ALU.subtract)
            # auxf[:, 0:2C] <- frac = r - floor
            nc.vector.tensor_tensor(
                auxf[:, 0 : 2 * CHUNK], auxf[:, 0 : 2 * CHUNK], r0f[:], ALU.subtract
            )

            # ---- flat offset F = PITCH*fy + fx  (f32, exact ints) ----
            Ff = spool.tile([9, CHUNK], F32, tag="Ff")
            nc.vector.tensor_scalar(
                Ff[:], r0f[:, 0:CHUNK], float(PITCH), None, op0=ALU.mult
            )
            nc.vector.tensor_tensor(Ff[:], Ff[:], r0f[:, CHUNK : 2 * CHUNK], ALU.add)

            # ---- wrapped gather indices (f32 math, one int16 convert) ----
            idxw = ipool.tile([128, NIW], F32, tag="idxw", bufs=1)
            for k in range(KK):
                nc.sync.dma_start(
                    idxw[0:16, k * SW : (k + 1) * SW],
                    Ff[k : k + 1, :].rearrange("o (p s) -> o p s", p=16),
                )
            nc.sync.dma_start(idxw[16:32, :], idxw[0:16, :])
            nc.sync.dma_start(idxw[32:64, :], idxw[0:32, :])
            nc.sync.dma_start(idxw[64:128, :], idxw[0:64, :])
            idx0 = ipool.tile([128, NIW], F32, tag="idx0", bufs=1)
            nc.vector.tensor_tensor(idx0[:], idxw[:], t_sprime[:], ALU.add)
            idxf = ipool.tile([128, 4 * NIW], F32, tag="idxf", bufs=1)
            for q in range(4):
                nc.vector.tensor_scalar(
                    idxf[:, q * NIW : (q + 1) * NIW],
                    idx0[:],
                    float(t * CHUNK + CORNER_OFF[q]),
                    None,
                    op0=ALU.add,
                )
            nc.vector.tensor_scalar(
                idxf[:], idxf[:], 0.0, float(XWN - 1), op0=ALU.max, op1=ALU.min
            )
            idxt = ipool.tile([128, 4 * NIW], I16, tag="idxt")
            nc.vector.tensor_copy(idxt[:], idxf[:])

            # ---- corner weights (mask folded in), written in wrapped order ----
            ly = auxf[:, 0:CHUNK]
            lx = auxf[:, CHUNK : 2 * CHUNK]
            msk = auxf[:, 2 * CHUNK : 3 * CHUNK]
            t1 = spool.tile([9, CHUNK], F32, tag="t1")   # 1-ly
            nc.vector.tensor_scalar(t1[:], ly, -1.0, 1.0, op0=ALU.mult, op1=ALU.add)
            t2 = spool.tile([9, CHUNK], F32, tag="t2")   # 1-lx
            nc.vector.tensor_scalar(t2[:], lx, -1.0, 1.0, op0=ALU.mult, op1=ALU.add)
            Aw = spool.tile([9, CHUNK], F32, tag="Ff")   # m*(1-ly), reuse slot
            nc.vector.tensor_tensor(Aw[:], t1[:], msk, ALU.mult)
            Bw = spool.tile([9, CHUNK], F32, tag="t1")   # m*ly, reuse slot
            nc.vector.tensor_tensor(Bw[:], ly, msk, ALU.mult)
            wrows = wrow_pool.tile([9, 4 * CHUNK], BF16, tag="wrows")

            def wwrap(q):
                # wrapped-order write view: input streams natural n'=54p+s,
                # output lands at j = 16s+p
                return wrows[:, q * CHUNK : (q + 1) * CHUNK].rearrange(
                    "c (s p) -> c p s", p=16
                )

            def nat(ap):
                # matching [9, 16, 54] natural-order read view
                return ap.rearrange("c (p s) -> c p s", p=16)

            nc.vector.tensor_tensor(wwrap(0), nat(Aw[:]), nat(t2[:]), ALU.mult)
            nc.vector.tensor_tensor(wwrap(1), nat(Aw[:]), nat(lx), ALU.mult)
            nc.vector.tensor_tensor(wwrap(2), nat(Bw[:]), nat(t2[:]), ALU.mult)
            nc.vector.tensor_tensor(wwrap(3), nat(Bw[:]), nat(lx), ALU.mult)

            # ---- per tap: gather corners, broadcast weights, combine, matmul ----
            pm_tiles = {}
            for k in range(KK):
                # broadcast tap-k corner-weight row to all 128 partitions
                # with a single gpsimd op (no DMA ladder)
                stg = wstg_pool.tile([1, 4 * CHUNK], BF16, tag="stg")
                nc.sync.dma_start(stg[:], wrows[k : k + 1, :])
                wb = wbpool.tile([128, 4 * CHUNK], BF16, tag="wb")
                nc.gpsimd.partition_broadcast(wb[:], stg[:], channels=128)

                g4 = gpool.tile([128, 4 * CHUNK], F32, tag="g4")
                for q in range(4):
                    nc.gpsimd.ap_gather(
                        g4[:, q * CHUNK : (q + 1) * CHUNK],
                        t_xw[:],
                        idxt[:, q * NIW + k * SW : q * NIW + (k + 1) * SW],
                        channels=128,
                        num_elems=XWN,
                        d=1,
                        num_idxs=CHUNK,
                    )
                # weighted combine -> bf16 val for the bf16 main matmul
                wg = vpool.tile([128, 2 * CHUNK], BF16, tag="wg", bufs=1)
                nc.vector.tensor_tensor(
                    wg[:], g4[:, 0 : 2 * CHUNK], wb[:, 0 : 2 * CHUNK], ALU.mult
                )
                wg2 = vpool.tile([128, 2 * CHUNK], BF16, tag="wg2", bufs=1)
                nc.vector.tensor_tensor(
                    wg2[:], g4[:, 2 * CHUNK : 4 * CHUNK], wb[:, 2 * CHUNK :], ALU.mult
                )
                val = vpool.tile([128, CHUNK], BF16, tag="val")
                nc.vector.tensor_tensor(
                    val[:],
                    wg[:, 0:CHUNK],
                    wg[:, CHUNK : 2 * CHUNK],
                    ALU.add,
                )
                nc.vector.tensor_tensor(val[:], val[:], wg2[:, 0:CHUNK], ALU.add)
                nc.vector.tensor_tensor(
                    val[:], val[:], wg2[:, CHUNK : 2 * CHUNK], ALU.add
                )

                for cb in range(2):
                    for u in range(SUBS):
                        if k == 0:
                            pm = mpool.tile([128, SUB], F32, tag=f"mp{cb}{u}")
                            pm_tiles[(cb, u)] = pm
                        nc.tensor.matmul(
                            pm_tiles[(cb, u)][:],
                            t_wmain[:, k * CO + cb * 128 : k * CO + cb * 128 + 128],
                            val[:, u * SUB : (u + 1) * SUB],
                            start=(k == 0),
                            stop=(k == KK - 1),
                        )

            # ---- bias + evict (vector keeps scalar queue free) + unwrap ----
            for cb in range(2):
                osb = opool.tile([128, CHUNK], F32, tag=f"osb{cb}", name=f"osb{cb}")
                for u in range(SUBS):
                    nc.vector.tensor_scalar(
                        osb[:, u * SUB : (u + 1) * SUB],
                        pm_tiles[(cb, u)][:],
                        t_bmain[:, cb : cb + 1],
                        None,
                        op0=ALU.add,
                    )
                unw = upool.tile([128, CHUNK], F32, tag=f"unw{cb}", name=f"unw{cb}")
                # read j = 16s+p while iterating (p, s) -> natural n' = 54p+s
                nc.vector.tensor_copy(
                    unw[:].rearrange("c (p s) -> c p s", p=16),
                    osb[:].rearrange("c (s p) -> c p s", p=16),
                )
                nc.sync.dma_start(
                    d_out[cb * 128 : (cb + 1) * 128, RPC * t : RPC * (t + 1), :],
                    unw[:].rearrange("c (h w) -> c h w", w=PITCH)[:, :, PAD : PAD + W],
                )

    nc.compile()
    return nc


def _host_inputs(x, weight, bias, offset_w, offset_b, mod_w, mod_b):
    """Build the 8 per-core input maps."""
    # main conv lhsT: wmain[c, k*256+o] = weight[o, c, ky, kx]
    wmain = np.ascontiguousarray(
        weight.reshape(CO, C, KK).transpose(1, 2, 0).reshape(C, KK * CO)
    ).astype(bfloat16)

    # aux channel order: j<9 dy_j (= offset ch 2j), j<18 dx, j<27 mask
    waux = np.zeros((C, KK, 27), np.float32)
    ow = offset_w.reshape(18, C, KK)
    mw = mod_w.reshape(9, C, KK)
    for j in range(9):
        waux[:, :, j] = ow[2 * j]
        waux[:, :, 9 + j] = ow[2 * j + 1]
        waux[:, :, 18 + j] = mw[j]
    waux = np.ascontiguousarray(waux.reshape(C, KK * 27)).astype(bfloat16)

    jj = np.arange(9)
    kyc = ((jj // 3) - 1).astype(np.float32) + offset_b[2 * jj]
    kxc = ((jj % 3) - 1).astype(np.float32) + offset_b[2 * jj + 1]
    baux = np.concatenate([kyc, kxc, mod_b.astype(np.float32)]).reshape(27, 1)
    bmain = np.stack([bias[:128], bias[128:]], axis=1).astype(np.float32)

    p = np.arange(128)
    scol = np.arange(KK * SW) % SW
    sprime = (IDX_BASE + SW * (p[:, None] % 16) + scol[None, :]).astype(np.float32)

    xpad = np.pad(x, ((0, 0), (0, 0), (PAD, PAD), (PAD, PAD)))  # [B,C,108,108]
    in_maps = []
    for core in range(8):
        b, half = core // 2, core % 2
        xw = np.ascontiguousarray(
            xpad[b, :, half * HR : half * HR + ROWS, :].reshape(C, XWN)
        ).astype(np.float32)
        in_maps.append(
            {
                "xw": xw,
                "xwb": xw.astype(bfloat16),
                "wmain": wmain,
                "waux": waux,
                "baux": baux,
                "bmain": bmain,
                "sprime": sprime,
            }
        )
    return in_maps


def get_program():
    if "nc" not in _CACHE:
        _CACHE["nc"] = _build_program()
    return _CACHE["nc"]


def kernel(**inputs):
    nc = get_program()
    in_maps = _host_inputs(
        np.asarray(inputs["x"], np.float32),
        np.asarray(inputs["weight"], np.float32),
        np.asarray(inputs["bias"], np.float32),
        np.asarray(inputs["offset_w"], np.float32),
        np.asarray(inputs["offset_b"], np.float32),
        np.asarray(inputs["mod_w"], np.float32),
        np.asarray(inputs["mod_b"], np.float32),
    )
    trace = bool(os.environ.get("DCN_TRACE"))
    res = run_bass_kernel_spmd(
        nc,
        in_maps,
        core_ids=list(range(8)),
        trace=trace,
    )
    _CACHE["last_results"] = res
    out = np.empty((B, CO, H, W), np.float32)
    for core in range(8):
        b, half = core // 2, core % 2
        out[b, :, half * HR : (half + 1) * HR, :] = res.results[core]["out"]
    return out


# revision 12
# speedup vs baseline: 1.0348x; 1.0348x over previous
"""Deformable Conv2d (DCNv2) Trainium2 Bass kernel.

Sharding: 8 cores; core c handles batch b = c//2, output-row half c%2
(48 of 96 rows). Each core receives a zero-padded window of its batch's
input (60 rows x 108 cols, pad 6 each side) so all bilinear samples and
the aux 3x3 convs are core-local.

Column ordering: the aux pipeline (aux conv, offsets, corner weights,
flat gather indices) runs in natural raster order n'. GPSIMD ap_gather
consumes indices "wrapped" over 16 partitions (output column i takes
the index at partition i%16, slot i//16), so the gather/combine/main-
matmul stage runs in wrapped order j, where within a 864-column chunk
j = 16*s + p corresponds to n'_local = 54*p + s. Corner weights are
written through a wrap-permuting access pattern, index tiles are built
with contiguous-run DMAs + a doubling ladder, and outputs are unwrapped
with one strided copy before the store DMA.

Per-tap corner weights are broadcast to all 128 channel partitions with
one gpsimd partition_broadcast (instead of a serialized ladder of
doubling DMAs), the combined bilinear values are written as bf16, and
the main + aux convolutions run as bf16 matmuls (1 cycle/row vs fp32's
4). Main-PSUM eviction runs on the vector engine so the scalar queue
never blocks the next chunk's aux activations.
"""

import os
import sys
from contextlib import ExitStack

import numpy as np
from ml_dtypes import bfloat16

if "/opt/trn_rl_repo" not in sys.path:
    sys.path.insert(0, "/opt/trn_rl_repo")

import concourse.bass as bass
import concourse.bacc as bacc
import concourse.mybir as mybir
import concourse.tile as tile
from concourse.bass_utils import run_bass_kernel_spmd

F32 = mybir.dt.float32
BF16 = mybir.dt.bfloat16
I16 = mybir.dt.int16
I32 = mybir.dt.int32
ALU = mybir.AluOpType
ACTF = mybir.ActivationFunctionType

# problem shape (hardcoded)
B, C, CO, H, W = 4, 128, 256, 96, 96
KK = 9
PAD = 6               # window pad on each side
HR = 48               # output rows per core
ROWS = HR + 2 * PAD   # 60 window rows
PITCH = W + 2 * PAD   # 108
XWN = ROWS * PITCH    # 6480 window elems
NP = HR * PITCH       # 5184 pipeline columns (with junk cols)
SW = 54               # wrapped idx slots per gather call
CHUNK = 16 * SW       # 864
NCHUNK = NP // CHUNK  # 6
SUB = 432             # matmul N-tile
SUBS = CHUNK // SUB   # 2
RPC = CHUNK // PITCH  # 8 output rows per chunk
IDX_BASE = PAD * PITCH  # 648 (w_w already includes the column pad)
CORNER_OFF = (0, 1, PITCH, PITCH + 1)

_CACHE: dict = {}


def _conv_off(ky, kx):
    # window-flat offset of conv tap (ky,kx) relative to output column n'
    return (PAD - 1 + ky) * PITCH + (kx - 1)


def _build_program():
    nc = bacc.Bacc(
        "TRN2",
        target_bir_lowering=False,
        debug=False,
        enable_asserts=False,
        num_devices=1,
    )
    d_xw = nc.dram_tensor("xw", [C, XWN], F32, kind="ExternalInput").ap()
    d_xwb = nc.dram_tensor("xwb", [C, XWN], BF16, kind="ExternalInput").ap()
    d_wmain = nc.dram_tensor("wmain", [C, KK * CO], BF16, kind="ExternalInput").ap()
    d_waux = nc.dram_tensor("waux", [C, KK * 27], BF16, kind="ExternalInput").ap()
    d_baux = nc.dram_tensor("baux", [27, 1], F32, kind="ExternalInput").ap()
    d_bmain = nc.dram_tensor("bmain", [128, 2], F32, kind="ExternalInput").ap()
    d_sprime = nc.dram_tensor("sprime", [128, KK * SW], F32, kind="ExternalInput").ap()
    d_out = nc.dram_tensor("out", [CO, HR, W], F32, kind="ExternalOutput").ap()

    NIW = KK * SW  # idx cols per corner per chunk (486)

    with tile.TileContext(nc) as tc, ExitStack() as ctx:
        cpool = ctx.enter_context(tc.tile_pool(name="consts", bufs=1))
        t_xw = cpool.tile([C, XWN], F32, tag="xw")
        nc.sync.dma_start(t_xw[:], d_xw)
        t_xwb = cpool.tile([C, XWN], BF16, tag="xwb")
        nc.sync.dma_start(t_xwb[:], d_xwb)
        t_wmain = cpool.tile([C, KK * CO], BF16, tag="wmain")
        nc.sync.dma_start(t_wmain[:], d_wmain)
        t_waux = cpool.tile([C, KK * 27], BF16, tag="waux")
        nc.sync.dma_start(t_waux[:], d_waux)
        t_baux = cpool.tile([27, 1], F32, tag="baux")
        nc.sync.dma_start(t_baux[:], d_baux)
        t_bmain = cpool.tile([128, 2], F32, tag="bmain")
        nc.sync.dma_start(t_bmain[:], d_bmain)
        t_sprime = cpool.tile([128, NIW], F32, tag="sprime")
        nc.sync.dma_start(t_sprime[:], d_sprime)

        apool = ctx.enter_context(tc.tile_pool(name="auxp", bufs=2, space="PSUM"))
        mpool = ctx.enter_context(tc.tile_pool(name="mainp", bufs=1, space="PSUM"))
        auxf_pool = ctx.enter_context(tc.tile_pool(name="auxf", bufs=2))
        spool = ctx.enter_context(tc.tile_pool(name="scratch", bufs=1))
        wrow_pool = ctx.enter_context(tc.tile_pool(name="wrows", bufs=1))
        wbpool = ctx.enter_context(tc.tile_pool(name="wb", bufs=1))
        wstg_pool = ctx.enter_context(tc.tile_pool(name="wstg", bufs=2))
        ipool = ctx.enter_context(tc.tile_pool(name="idx", bufs=2))
        gpool = ctx.enter_context(tc.tile_pool(name="gath", bufs=2))
        vpool = ctx.enter_context(tc.tile_pool(name="val", bufs=2))
        opool = ctx.enter_context(tc.tile_pool(name="outsb", bufs=1))
        upool = ctx.enter_context(tc.tile_pool(name="unw", bufs=1))

        for t in range(NCHUNK):
            cbase = t * CHUNK

            # ---- aux conv: 27 channels over this chunk, natural order ----
            aux27 = auxf_pool.tile([27, CHUNK], F32, tag="aux27", bufs=1)
            for u in range(SUBS):
                pa = apool.tile([27, SUB], F32, tag="auxpsum")
                for k in range(KK):
                    ky, kx = k // 3, k % 3
                    base = cbase + u * SUB + _conv_off(ky, kx)
                    nc.tensor.matmul(
                        pa[:],
                        t_waux[:, k * 27 : (k + 1) * 27],
                        t_xwb[:, base : base + SUB],
                        start=(k == 0),
                        stop=(k == KK - 1),
                    )
                nc.scalar.activation(
                    aux27[:, u * SUB : (u + 1) * SUB],
                    pa[:],
                    ACTF.Identity,
                    bias=t_baux[:, 0:1],
                )
            # regroup the three 9-row bands side by side on partitions 0-8
            auxf = auxf_pool.tile([9, 3 * CHUNK], F32, tag="auxf", bufs=1)
            nc.sync.dma_start(auxf[:, 0:CHUNK], aux27[0:9, :])
            nc.sync.dma_start(auxf[:, CHUNK : 2 * CHUNK], aux27[9:18, :])
            nc.sync.dma_start(auxf[:, 2 * CHUNK : 3 * CHUNK], aux27[18:27, :])
            # mask = sigmoid(logit), in place at partition base 0
            nc.scalar.activation(
                auxf[:, 2 * CHUNK : 3 * CHUNK],
                auxf[:, 2 * CHUNK : 3 * CHUNK],
                ACTF.Sigmoid,
            )

            # ---- floor(ry), floor(rx); fractional parts ----
            c32 = spool.tile([9, 2 * CHUNK], I32, tag="c32")
            nc.vector.tensor_copy(c32[:], auxf[:, 0 : 2 * CHUNK])
            r0f = spool.tile([9, 2 * CHUNK], F32, tag="r0f")
            nc.vector.tensor_copy(r0f[:], c32[:])
            gt = spool.tile([9, 2 * CHUNK], F32, tag="c32")  # reuse slot
            nc.vector.tensor_tensor(gt[:], r0f[:], auxf[:, 0 : 2 * CHUNK], ALU.is_gt)
            # r0f <- floor = round - (round > x)
            nc.vector.tensor_tensor(r0f[:], r0f[:], gt[:], ALU.subtract)
            # auxf[:, 0:2C] <- frac = r - floor
            nc.vector.tensor_tensor(
                auxf[:, 0 : 2 * CHUNK], auxf[:, 0 : 2 * CHUNK], r0f[:], ALU.subtract
            )

            # ---- flat offset F = PITCH*fy + fx  (f32, exact ints) ----
            Ff = spool.tile([9, CHUNK], F32, tag="Ff")
            nc.vector.tensor_scalar(
                Ff[:], r0f[:, 0:CHUNK], float(PITCH), None, op0=ALU.mult
            )
            nc.vector.tensor_tensor(Ff[:], Ff[:], r0f[:, CHUNK : 2 * CHUNK], ALU.add)

            # ---- wrapped gather indices (f32 math, one int16 convert) ----
            idxw = ipool.tile([128, NIW], F32, tag="idxw", bufs=1)
            for k in range(KK):
                nc.sync.dma_start(
                    idxw[0:16, k * SW : (k + 1) * SW],
                    Ff[k : k + 1, :].rearrange("o (p s) -> o p s", p=16),
                )
            nc.sync.dma_start(idxw[16:32, :], idxw[0:16, :])
            nc.sync.dma_start(idxw[32:64, :], idxw[0:32, :])
            nc.sync.dma_start(idxw[64:128, :], idxw[0:64, :])
            idx0 = ipool.tile([128, NIW], F32, tag="idx0", bufs=1)
            nc.vector.tensor_tensor(idx0[:], idxw[:], t_sprime[:], ALU.add)
            idxf = ipool.tile([128, 4 * NIW], F32, tag="idxf", bufs=1)
            for q in range(4):
                nc.vector.tensor_scalar(
                    idxf[:, q * NIW : (q + 1) * NIW],
                    idx0[:],
                    float(t * CHUNK + CORNER_OFF[q]),
                    None,
                    op0=ALU.add,
                )
            nc.vector.tensor_scalar(
                idxf[:], idxf[:], 0.0, float(XWN - 1), op0=ALU.max, op1=ALU.min
            )
            idxt = ipool.tile([128, 4 * NIW], I16, tag="idxt", bufs=1)
            nc.vector.tensor_copy(idxt[:], idxf[:])

            # ---- corner weights (mask folded in), written in wrapped order ----
            ly = auxf[:, 0:CHUNK]
            lx = auxf[:, CHUNK : 2 * CHUNK]
            msk = auxf[:, 2 * CHUNK : 3 * CHUNK]
            t1 = spool.tile([9, CHUNK], F32, tag="t1")   # 1-ly
            nc.vector.tensor_scalar(t1[:], ly, -1.0, 1.0, op0=ALU.mult, op1=ALU.add)
            t2 = spool.tile([9, CHUNK], F32, tag="t2")   # 1-lx
            nc.vector.tensor_scalar(t2[:], lx, -1.0, 1.0, op0=ALU.mult, op1=ALU.add)
            Aw = spool.tile([9, CHUNK], F32, tag="Ff")   # m*(1-ly), reuse slot
            nc.vector.tensor_tensor(Aw[:], t1[:], msk, ALU.mult)
            Bw = spool.tile([9, CHUNK], F32, tag="t1")   # m*ly, reuse slot
            nc.vector.tensor_tensor(Bw[:], ly, msk, ALU.mult)
            wrows = wrow_pool.tile([9, 4 * CHUNK], BF16, tag="wrows")

            def wwrap(q):
                # wrapped-order write view: input streams natural n'=54p+s,
                # output lands at j = 16s+p
                return wrows[:, q * CHUNK : (q + 1) * CHUNK].rearrange(
                    "c (s p) -> c p s", p=16
                )

            def nat(ap):
                # matching [9, 16, 54] natural-order read view
                return ap.rearrange("c (p s) -> c p s", p=16)

            nc.vector.tensor_tensor(wwrap(0), nat(Aw[:]), nat(t2[:]), ALU.mult)
            nc.vector.tensor_tensor(wwrap(1), nat(Aw[:]), nat(lx), ALU.mult)
            nc.vector.tensor_tensor(wwrap(2), nat(Bw[:]), nat(t2[:]), ALU.mult)
            nc.vector.tensor_tensor(wwrap(3), nat(Bw[:]), nat(lx), ALU.mult)

            # ---- taps in groups of 3: batch the gpsimd broadcasts ahead
            #      of the gathers (fewer Q7 op switches, deeper pipeline) ----
            pm_tiles = {}
            for g in range(3):
                wbs = {}
                for k in range(3 * g, 3 * g + 3):
                    stg = wstg_pool.tile([1, 4 * CHUNK], BF16, tag="stg")
                    nc.sync.dma_start(stg[:], wrows[k : k + 1, :])
                    wb = wbpool.tile(
                        [128, 4 * CHUNK], BF16, tag=f"wb{k % 3}", name=f"wb{k % 3}"
                    )
                    nc.gpsimd.partition_broadcast(wb[:], stg[:], channels=128)
                    wbs[k] = wb
                for k in range(3 * g, 3 * g + 3):
                    wb = wbs[k]
                    g4 = gpool.tile([128, 4 * CHUNK], F32, tag="g4")
                    for q in range(4):
                        nc.gpsimd.ap_gather(
                            g4[:, q * CHUNK : (q + 1) * CHUNK],
                            t_xw[:],
                            idxt[:, q * NIW + k * SW : q * NIW + (k + 1) * SW],
                            channels=128,
                            num_elems=XWN,
                            d=1,
                            num_idxs=CHUNK,
                        )
                    # weighted combine -> bf16 val for the bf16 main matmul
                    wg = vpool.tile([128, 2 * CHUNK], BF16, tag="wg", bufs=1)
                    nc.vector.tensor_tensor(
                        wg[:], g4[:, 0 : 2 * CHUNK], wb[:, 0 : 2 * CHUNK], ALU.mult
                    )
                    wg2 = vpool.tile([128, 2 * CHUNK], BF16, tag="wg2", bufs=1)
                    nc.vector.tensor_tensor(
                        wg2[:],
                        g4[:, 2 * CHUNK : 4 * CHUNK],
                        wb[:, 2 * CHUNK :],
                        ALU.mult,
                    )
                    val = vpool.tile([128, CHUNK], BF16, tag="val")
                    nc.vector.tensor_tensor(
                        val[:],
                        wg[:, 0:CHUNK],
                        wg[:, CHUNK : 2 * CHUNK],
                        ALU.add,
                    )
                    nc.vector.tensor_tensor(val[:], val[:], wg2[:, 0:CHUNK], ALU.add)
                    nc.vector.tensor_tensor(
                        val[:], val[:], wg2[:, CHUNK : 2 * CHUNK], ALU.add
                    )

                    for cb in range(2):
                        for u in range(SUBS):
                            if k == 0:
                                pm = mpool.tile(
                                    [128, SUB], F32, tag=f"mp{cb}{u}", name=f"mp{cb}{u}"
                                )
                                pm_tiles[(cb, u)] = pm
                            nc.tensor.matmul(
                                pm_tiles[(cb, u)][:],
                                t_wmain[
                                    :, k * CO + cb * 128 : k * CO + cb * 128 + 128
                                ],
                                val[:, u * SUB : (u + 1) * SUB],
                                start=(k == 0),
                                stop=(k == KK - 1),
                            )

            # ---- bias + evict (vector keeps scalar queue free) + unwrap ----
            for cb in range(2):
                osb = opool.tile([128, CHUNK], F32, tag=f"osb{cb}", name=f"osb{cb}")
                for u in range(SUBS):
                    nc.vector.tensor_scalar(
                        osb[:, u * SUB : (u + 1) * SUB],
                        pm_tiles[(cb, u)][:],
                        t_bmain[:, cb : cb + 1],
                        None,
                        op0=ALU.add,
                    )
                unw = upool.tile([128, CHUNK], F32, tag=f"unw{cb}", name=f"unw{cb}")
                # read j = 16s+p while iterating (p, s) -> natural n' = 54p+s
                nc.vector.tensor_copy(
                    unw[:].rearrange("c (p s) -> c p s", p=16),
                    osb[:].rearrange("c (s p) -> c p s", p=16),
                )
                nc.sync.dma_start(
                    d_out[cb * 128 : (cb + 1) * 128, RPC * t : RPC * (t + 1), :],
                    unw[:].rearrange("c (h w) -> c h w", w=PITCH)[:, :, PAD : PAD + W],
                )

    nc.compile()
    return nc


def _host_inputs(x, weight, bias, offset_w, offset_b, mod_w, mod_b):
    """Build the 8 per-core input maps."""
    # main conv lhsT: wmain[c, k*256+o] = weight[o, c, ky, kx]
    wmain = np.ascontiguousarray(
        weight.reshape(CO, C, KK).transpose(1, 2, 0).reshape(C, KK * CO)
    ).astype(bfloat16)

    # aux channel order: j<9 dy_j (= offset ch 2j), j<18 dx, j<27 mask
    waux = np.zeros((C, KK, 27), np.float32)
    ow = offset_w.reshape(18, C, KK)
    mw = mod_w.reshape(9, C, KK)
    for j in range(9):
        waux[:, :, j] = ow[2 * j]
        waux[:, :, 9 + j] = ow[2 * j + 1]
        waux[:, :, 18 + j] = mw[j]
    waux = np.ascontiguousarray(waux.reshape(C, KK * 27)).astype(bfloat16)

    jj = np.arange(9)
    kyc = ((jj // 3) - 1).astype(np.float32) + offset_b[2 * jj]
    kxc = ((jj % 3) - 1).astype(np.float32) + offset_b[2 * jj + 1]
    baux = np.concatenate([kyc, kxc, mod_b.astype(np.float32)]).reshape(27, 1)
    bmain = np.stack([bias[:128], bias[128:]], axis=1).astype(np.float32)

    p = np.arange(128)
    scol = np.arange(KK * SW) % SW
    sprime = (IDX_BASE + SW * (p[:, None] % 16) + scol[None, :]).astype(np.float32)

    xpad = np.pad(x, ((0, 0), (0, 0), (PAD, PAD), (PAD, PAD)))  # [B,C,108,108]
    in_maps = []
    for core in range(8):
        b, half = core // 2, core % 2
        xw = np.ascontiguousarray(
            xpad[b, :, half * HR : half * HR + ROWS, :].reshape(C, XWN)
        ).astype(np.float32)
        in_maps.append(
            {
                "xw": xw,
                "xwb": xw.astype(bfloat16),
                "wmain": wmain,
                "waux": waux,
                "baux": baux,
                "bmain": bmain,
                "sprime": sprime,
            }
        )
    return in_maps


def get_program():
    if "nc" not in _CACHE:
        _CACHE["nc"] = _build_program()
    return _CACHE["nc"]


def kernel(**inputs):
    nc = get_program()
    in_maps = _host_inputs(
        np.asarray(inputs["x"], np.float32),
        np.asarray(inputs["weight"], np.float32),
        np.asarray(inputs["bias"], np.float32),
        np.asarray(inputs["offset_w"], np.float32),
        np.asarray(inputs["offset_b"], np.float32),
        np.asarray(inputs["mod_w"], np.float32),
        np.asarray(inputs["mod_b"], np.float32),
    )
    trace = bool(os.environ.get("DCN_TRACE"))
    res = run_bass_kernel_spmd(
        nc,
        in_maps,
        core_ids=list(range(8)),
        trace=trace,
    )
    _CACHE["last_results"] = res
    out = np.empty((B, CO, H, W), np.float32)
    for core in range(8):
        b, half = core // 2, core % 2
        out[b, :, half * HR : (half + 1) * HR, :] = res.results[core]["out"]
    return out


# revision 13
# speedup vs baseline: 1.1321x; 1.0940x over previous
"""Deformable Conv2d (DCNv2) Trainium2 Bass kernel.

Sharding: 8 cores; core c handles batch b = c//2, output-row half c%2
(48 of 96 rows). Each core receives a zero-padded window of its batch's
input (60 rows x 108 cols, pad 6 each side) so all bilinear samples and
the aux 3x3 convs are core-local.

Column ordering: the aux pipeline (aux conv, offsets, corner weights,
flat gather indices) runs in natural raster order n'. GPSIMD ap_gather
consumes indices "wrapped" over 16 partitions (output column i takes
the index at partition i%16, slot i//16), so the gather/combine/main-
matmul stage runs in wrapped order j, where within a 864-column chunk
j = 16*s + p corresponds to n'_local = 54*p + s. Corner weights are
written through a wrap-permuting access pattern, index tiles are built
with contiguous-run DMAs + a doubling ladder, and outputs are unwrapped
with one strided copy before the store DMA.

Per-tap corner weights are broadcast to all 128 channel partitions with
one gpsimd partition_broadcast (instead of a serialized ladder of
doubling DMAs), the combined bilinear values are written as bf16, and
the main + aux convolutions run as bf16 matmuls (1 cycle/row vs fp32's
4). Main-PSUM eviction runs on the vector engine so the scalar queue
never blocks the next chunk's aux activations.
"""

import os
import sys
from contextlib import ExitStack

import numpy as np
from ml_dtypes import bfloat16

if "/opt/trn_rl_repo" not in sys.path:
    sys.path.insert(0, "/opt/trn_rl_repo")

import concourse.bass as bass
import concourse.bacc as bacc
import concourse.mybir as mybir
import concourse.tile as tile
from concourse.bass_utils import run_bass_kernel_spmd

F32 = mybir.dt.float32
BF16 = mybir.dt.bfloat16
I16 = mybir.dt.int16
I32 = mybir.dt.int32
ALU = mybir.AluOpType
ACTF = mybir.ActivationFunctionType

# problem shape (hardcoded)
B, C, CO, H, W = 4, 128, 256, 96, 96
KK = 9
PAD = 6               # window pad on each side
HR = 48               # output rows per core
ROWS = HR + 2 * PAD   # 60 window rows
PITCH = W + 2 * PAD   # 108
XWN = ROWS * PITCH    # 6480 window elems
NP = HR * PITCH       # 5184 pipeline columns (with junk cols)
SW = 54               # wrapped idx slots per gather call
CHUNK = 16 * SW       # 864
NCHUNK = NP // CHUNK  # 6
SUB = 432             # matmul N-tile
SUBS = CHUNK // SUB   # 2
RPC = CHUNK // PITCH  # 8 output rows per chunk
IDX_BASE = PAD * PITCH  # 648 (w_w already includes the column pad)
CORNER_OFF = (0, 1, PITCH, PITCH + 1)

_CACHE: dict = {}


def _conv_off(ky, kx):
    # window-flat offset of conv tap (ky,kx) relative to output column n'
    return (PAD - 1 + ky) * PITCH + (kx - 1)


def _build_program():
    nc = bacc.Bacc(
        "TRN2",
        target_bir_lowering=False,
        debug=False,
        enable_asserts=False,
        num_devices=1,
    )
    d_xw = nc.dram_tensor("xw", [C, XWN], F32, kind="ExternalInput").ap()
    d_xwb = nc.dram_tensor("xwb", [C, XWN], BF16, kind="ExternalInput").ap()
    d_wmain = nc.dram_tensor("wmain", [C, KK * CO], BF16, kind="ExternalInput").ap()
    d_waux = nc.dram_tensor("waux", [C, KK * 27], BF16, kind="ExternalInput").ap()
    d_baux = nc.dram_tensor("baux", [27, 1], F32, kind="ExternalInput").ap()
    d_bmain = nc.dram_tensor("bmain", [128, 2], F32, kind="ExternalInput").ap()
    d_sprime = nc.dram_tensor("sprime", [128, KK * SW], F32, kind="ExternalInput").ap()
    d_out = nc.dram_tensor("out", [CO, HR, W], F32, kind="ExternalOutput").ap()

    NIW = KK * SW  # idx cols per corner per chunk (486)

    with tile.TileContext(nc) as tc, ExitStack() as ctx:
        cpool = ctx.enter_context(tc.tile_pool(name="consts", bufs=1))
        t_xw = cpool.tile([C, XWN], F32, tag="xw")
        nc.sync.dma_start(t_xw[:], d_xw)
        t_xwb = cpool.tile([C, XWN], BF16, tag="xwb")
        nc.sync.dma_start(t_xwb[:], d_xwb)
        t_wmain = cpool.tile([C, KK * CO], BF16, tag="wmain")
        nc.sync.dma_start(t_wmain[:], d_wmain)
        t_waux = cpool.tile([C, KK * 27], BF16, tag="waux")
        nc.sync.dma_start(t_waux[:], d_waux)
        t_baux = cpool.tile([27, 1], F32, tag="baux")
        nc.sync.dma_start(t_baux[:], d_baux)
        t_bmain = cpool.tile([128, 2], F32, tag="bmain")
        nc.sync.dma_start(t_bmain[:], d_bmain)
        t_sprime = cpool.tile([128, NIW], F32, tag="sprime")
        nc.sync.dma_start(t_sprime[:], d_sprime)

        apool = ctx.enter_context(tc.tile_pool(name="auxp", bufs=2, space="PSUM"))
        mpool = ctx.enter_context(tc.tile_pool(name="mainp", bufs=1, space="PSUM"))
        auxf_pool = ctx.enter_context(tc.tile_pool(name="auxf", bufs=2))
        spool = ctx.enter_context(tc.tile_pool(name="scratch", bufs=1))
        wrow_pool = ctx.enter_context(tc.tile_pool(name="wrows", bufs=1))
        wbpool = ctx.enter_context(tc.tile_pool(name="wb", bufs=1))
        wstg_pool = ctx.enter_context(tc.tile_pool(name="wstg", bufs=2))
        ipool = ctx.enter_context(tc.tile_pool(name="idx", bufs=2))
        gpool = ctx.enter_context(tc.tile_pool(name="gath", bufs=2))
        vpool = ctx.enter_context(tc.tile_pool(name="val", bufs=2))
        opool = ctx.enter_context(tc.tile_pool(name="outsb", bufs=1))
        upool = ctx.enter_context(tc.tile_pool(name="unw", bufs=1))

        for t in range(NCHUNK):
            cbase = t * CHUNK

            # ---- aux conv: 27 channels over this chunk, natural order ----
            aux27 = auxf_pool.tile([27, CHUNK], F32, tag="aux27", bufs=1)
            for u in range(SUBS):
                pa = apool.tile([27, SUB], F32, tag="auxpsum")
                for k in range(KK):
                    ky, kx = k // 3, k % 3
                    base = cbase + u * SUB + _conv_off(ky, kx)
                    nc.tensor.matmul(
                        pa[:],
                        t_waux[:, k * 27 : (k + 1) * 27],
                        t_xwb[:, base : base + SUB],
                        start=(k == 0),
                        stop=(k == KK - 1),
                    )
                nc.scalar.activation(
                    aux27[:, u * SUB : (u + 1) * SUB],
                    pa[:],
                    ACTF.Identity,
                    bias=t_baux[:, 0:1],
                )
            # regroup the three 9-row bands side by side on partitions 0-8
            auxf = auxf_pool.tile([9, 3 * CHUNK], F32, tag="auxf", bufs=1)
            nc.sync.dma_start(auxf[:, 0:CHUNK], aux27[0:9, :])
            nc.sync.dma_start(auxf[:, CHUNK : 2 * CHUNK], aux27[9:18, :])
            nc.sync.dma_start(auxf[:, 2 * CHUNK : 3 * CHUNK], aux27[18:27, :])
            # mask = sigmoid(logit), in place at partition base 0
            nc.scalar.activation(
                auxf[:, 2 * CHUNK : 3 * CHUNK],
                auxf[:, 2 * CHUNK : 3 * CHUNK],
                ACTF.Sigmoid,
            )

            # ---- floor(ry), floor(rx); fractional parts ----
            c32 = spool.tile([9, 2 * CHUNK], I32, tag="c32")
            nc.vector.tensor_copy(c32[:], auxf[:, 0 : 2 * CHUNK])
            r0f = spool.tile([9, 2 * CHUNK], F32, tag="r0f")
            nc.vector.tensor_copy(r0f[:], c32[:])
            gt = spool.tile([9, 2 * CHUNK], F32, tag="c32")  # reuse slot
            nc.vector.tensor_tensor(gt[:], r0f[:], auxf[:, 0 : 2 * CHUNK], ALU.is_gt)
            # r0f <- floor = round - (round > x)
            nc.vector.tensor_tensor(r0f[:], r0f[:], gt[:], ALU.subtract)
            # auxf[:, 0:2C] <- frac = r - floor
            nc.vector.tensor_tensor(
                auxf[:, 0 : 2 * CHUNK], auxf[:, 0 : 2 * CHUNK], r0f[:], ALU.subtract
            )

            # ---- flat offset F = PITCH*fy + fx  (f32, exact ints) ----
            Ff = spool.tile([9, CHUNK], F32, tag="Ff")
            nc.vector.tensor_scalar(
                Ff[:], r0f[:, 0:CHUNK], float(PITCH), None, op0=ALU.mult
            )
            nc.vector.tensor_tensor(Ff[:], Ff[:], r0f[:, CHUNK : 2 * CHUNK], ALU.add)

            # ---- wrapped gather indices (f32 math, one int16 convert) ----
            idxw = ipool.tile([128, NIW], F32, tag="idxw", bufs=1)
            for k in range(KK):
                nc.sync.dma_start(
                    idxw[0:16, k * SW : (k + 1) * SW],
                    Ff[k : k + 1, :].rearrange("o (p s) -> o p s", p=16),
                )
            nc.sync.dma_start(idxw[16:32, :], idxw[0:16, :])
            nc.sync.dma_start(idxw[32:64, :], idxw[0:32, :])
            nc.sync.dma_start(idxw[64:128, :], idxw[0:64, :])
            idx0 = ipool.tile([128, NIW], F32, tag="idx0", bufs=1)
            nc.vector.tensor_tensor(idx0[:], idxw[:], t_sprime[:], ALU.add)
            idxf = ipool.tile([128, 4 * NIW], F32, tag="idxf", bufs=1)
            for q in range(4):
                nc.vector.tensor_scalar(
                    idxf[:, q * NIW : (q + 1) * NIW],
                    idx0[:],
                    float(t * CHUNK + CORNER_OFF[q]),
                    None,
                    op0=ALU.add,
                )
            nc.vector.tensor_scalar(
                idxf[:], idxf[:], 0.0, float(XWN - 1), op0=ALU.max, op1=ALU.min
            )
            idxt = ipool.tile([128, 4 * NIW], I16, tag="idxt", bufs=1)
            nc.vector.tensor_copy(idxt[:], idxf[:])

            # ---- corner weights (mask folded in), written in wrapped order ----
            ly = auxf[:, 0:CHUNK]
            lx = auxf[:, CHUNK : 2 * CHUNK]
            msk = auxf[:, 2 * CHUNK : 3 * CHUNK]
            t1 = spool.tile([9, CHUNK], F32, tag="t1")   # 1-ly
            nc.vector.tensor_scalar(t1[:], ly, -1.0, 1.0, op0=ALU.mult, op1=ALU.add)
            t2 = spool.tile([9, CHUNK], F32, tag="t2")   # 1-lx
            nc.vector.tensor_scalar(t2[:], lx, -1.0, 1.0, op0=ALU.mult, op1=ALU.add)
            Aw = spool.tile([9, CHUNK], F32, tag="Ff")   # m*(1-ly), reuse slot
            nc.vector.tensor_tensor(Aw[:], t1[:], msk, ALU.mult)
            Bw = spool.tile([9, CHUNK], F32, tag="t1")   # m*ly, reuse slot
            nc.vector.tensor_tensor(Bw[:], ly, msk, ALU.mult)
            wrows = wrow_pool.tile([9, 4 * CHUNK], BF16, tag="wrows")

            def wwrap(q):
                # wrapped-order write view: input streams natural n'=54p+s,
                # output lands at j = 16s+p
                return wrows[:, q * CHUNK : (q + 1) * CHUNK].rearrange(
                    "c (s p) -> c p s", p=16
                )

            def nat(ap):
                # matching [9, 16, 54] natural-order read view
                return ap.rearrange("c (p s) -> c p s", p=16)

            nc.vector.tensor_tensor(wwrap(0), nat(Aw[:]), nat(t2[:]), ALU.mult)
            nc.vector.tensor_tensor(wwrap(1), nat(Aw[:]), nat(lx), ALU.mult)
            nc.vector.tensor_tensor(wwrap(2), nat(Bw[:]), nat(t2[:]), ALU.mult)
            nc.vector.tensor_tensor(wwrap(3), nat(Bw[:]), nat(lx), ALU.mult)

            # ---- per tap: ladder-broadcast weights (bf16), gather,
            #      combine, matmul ----
            pm_tiles = {}
            wb8 = None
            for k in range(KK):
                if k % 2 == 0:
                    wid = 4 * CHUNK if k == KK - 1 else 8 * CHUNK
                    nrow = 1 if k == KK - 1 else 2
                    wb8 = wbpool.tile([128, 8 * CHUNK], BF16, tag="wb8")
                    eng = nc.scalar if (k % 4 == 0) else nc.sync
                    eng.dma_start(
                        wb8[0:1, 0:wid].rearrange("o (a b) -> o a b", a=nrow),
                        wrows[k : k + nrow, :],
                    )
                    eng.dma_start(wb8[1:2, 0:wid], wb8[0:1, 0:wid])
                    eng.dma_start(wb8[2:4, 0:wid], wb8[0:2, 0:wid])
                    eng.dma_start(wb8[4:8, 0:wid], wb8[0:4, 0:wid])
                    eng.dma_start(wb8[8:16, 0:wid], wb8[0:8, 0:wid])
                    eng.dma_start(wb8[16:32, 0:wid], wb8[0:16, 0:wid])
                    eng.dma_start(wb8[32:64, 0:wid], wb8[0:32, 0:wid])
                    eng.dma_start(wb8[64:128, 0:wid], wb8[0:64, 0:wid])
                wboff = (k % 2) * 4 * CHUNK

                g4 = gpool.tile([128, 4 * CHUNK], F32, tag="g4")
                for q in range(4):
                    nc.gpsimd.ap_gather(
                        g4[:, q * CHUNK : (q + 1) * CHUNK],
                        t_xw[:],
                        idxt[:, q * NIW + k * SW : q * NIW + (k + 1) * SW],
                        channels=128,
                        num_elems=XWN,
                        d=1,
                        num_idxs=CHUNK,
                    )
                # weighted combine -> bf16 val for the bf16 main matmul
                wg = vpool.tile([128, 2 * CHUNK], BF16, tag="wg", bufs=1)
                nc.vector.tensor_tensor(
                    wg[:],
                    g4[:, 0 : 2 * CHUNK],
                    wb8[:, wboff : wboff + 2 * CHUNK],
                    ALU.mult,
                )
                wg2 = vpool.tile([128, 2 * CHUNK], BF16, tag="wg2", bufs=1)
                nc.vector.tensor_tensor(
                    wg2[:],
                    g4[:, 2 * CHUNK : 4 * CHUNK],
                    wb8[:, wboff + 2 * CHUNK : wboff + 4 * CHUNK],
                    ALU.mult,
                )
                val = vpool.tile([128, CHUNK], BF16, tag="val")
                nc.vector.tensor_tensor(
                    val[:], wg[:, 0:CHUNK], wg[:, CHUNK : 2 * CHUNK], ALU.add
                )
                nc.vector.tensor_tensor(val[:], val[:], wg2[:, 0:CHUNK], ALU.add)
                nc.vector.tensor_tensor(
                    val[:], val[:], wg2[:, CHUNK : 2 * CHUNK], ALU.add
                )

                for cb in range(2):
                    for u in range(SUBS):
                        if k == 0:
                            pm = mpool.tile(
                                [128, SUB], F32, tag=f"mp{cb}{u}", name=f"mp{cb}{u}"
                            )
                            pm_tiles[(cb, u)] = pm
                        nc.tensor.matmul(
                            pm_tiles[(cb, u)][:],
                            t_wmain[:, k * CO + cb * 128 : k * CO + cb * 128 + 128],
                            val[:, u * SUB : (u + 1) * SUB],
                            start=(k == 0),
                            stop=(k == KK - 1),
                        )

            # ---- bias + evict (vector keeps scalar queue free) + unwrap ----
            for cb in range(2):
                osb = opool.tile([128, CHUNK], F32, tag=f"osb{cb}", name=f"osb{cb}")
                for u in range(SUBS):
                    nc.vector.tensor_scalar(
                        osb[:, u * SUB : (u + 1) * SUB],
                        pm_tiles[(cb, u)][:],
                        t_bmain[:, cb : cb + 1],
                        None,
                        op0=ALU.add,
                    )
                unw = upool.tile([128, CHUNK], F32, tag=f"unw{cb}", name=f"unw{cb}")
                # read j = 16s+p while iterating (p, s) -> natural n' = 54p+s
                nc.vector.tensor_copy(
                    unw[:].rearrange("c (p s) -> c p s", p=16),
                    osb[:].rearrange("c (s p) -> c p s", p=16),
                )
                nc.sync.dma_start(
                    d_out[cb * 128 : (cb + 1) * 128, RPC * t : RPC * (t + 1), :],
                    unw[:].rearrange("c (h w) -> c h w", w=PITCH)[:, :, PAD : PAD + W],
                )

    nc.compile()
    return nc


def _host_inputs(x, weight, bias, offset_w, offset_b, mod_w, mod_b):
    """Build the 8 per-core input maps."""
    # main conv lhsT: wmain[c, k*256+o] = weight[o, c, ky, kx]
    wmain = np.ascontiguousarray(
        weight.reshape(CO, C, KK).transpose(1, 2, 0).reshape(C, KK * CO)
    ).astype(bfloat16)

    # aux channel order: j<9 dy_j (= offset ch 2j), j<18 dx, j<27 mask
    waux = np.zeros((C, KK, 27), np.float32)
    ow = offset_w.reshape(18, C, KK)
    mw = mod_w.reshape(9, C, KK)
    for j in range(9):
        waux[:, :, j] = ow[2 * j]
        waux[:, :, 9 + j] = ow[2 * j + 1]
        waux[:, :, 18 + j] = mw[j]
    waux = np.ascontiguousarray(waux.reshape(C, KK * 27)).astype(bfloat16)

    jj = np.arange(9)
    kyc = ((jj // 3) - 1).astype(np.float32) + offset_b[2 * jj]
    kxc = ((jj % 3) - 1).astype(np.float32) + offset_b[2 * jj + 1]
    baux = np.concatenate([kyc, kxc, mod_b.astype(np.float32)]).reshape(27, 1)
    bmain = np.stack([bias[:128], bias[128:]], axis=1).astype(np.float32)

    p = np.arange(128)
    scol = np.arange(KK * SW) % SW
    sprime = (IDX_BASE + SW * (p[:, None] % 16) + scol[None, :]).astype(np.float32)

    xpad = np.pad(x, ((0, 0), (0, 0), (PAD, PAD), (PAD, PAD)))  # [B,C,108,108]
    in_maps = []
    for core in range(8):
        b, half = core // 2, core % 2
        xw = np.ascontiguousarray(
            xpad[b, :, half * HR : half * HR + ROWS, :].reshape(C, XWN)
        ).astype(np.float32)
        in_maps.append(
            {
                "xw": xw,
                "xwb": xw.astype(bfloat16),
                "wmain": wmain,
                "waux": waux,
                "baux": baux,
                "bmain": bmain,
                "sprime": sprime,
            }
        )
    return in_maps


def get_program():
    if "nc" not in _CACHE:
        _CACHE["nc"] = _build_program()
    return _CACHE["nc"]


def kernel(**inputs):
    nc = get_program()
    in_maps = _host_inputs(
        np.asarray(inputs["x"], np.float32),
        np.asarray(inputs["weight"], np.float32),
        np.asarray(inputs["bias"], np.float32),
        np.asarray(inputs["offset_w"], np.float32),
        np.asarray(inputs["offset_b"], np.float32),
        np.asarray(inputs["mod_w"], np.float32),
        np.asarray(inputs["mod_b"], np.float32),
    )
    trace = bool(os.environ.get("DCN_TRACE"))
    res = run_bass_kernel_spmd(
        nc,
        in_maps,
        core_ids=list(range(8)),
        trace=trace,
    )
    _CACHE["last_results"] = res
    out = np.empty((B, CO, H, W), np.float32)
    for core in range(8):
        b, half = core // 2, core % 2
        out[b, :, half * HR : (half + 1) * HR, :] = res.results[core]["out"]
    return out
